# revision 2
# baseline (speedup 1.0000x reference)
"""BEVNet kernel for trn2 (8 NeuronCores).

Current implementation: the full network expressed in JAX, jit-compiled for
the NeuronCore devices (XLA-Neuron backend), executed on the attached trn2
cores. Inputs are taken as full (unsharded) numpy arrays; outputs are full
numpy arrays. The heavy compute (convs / gathers / segment-max) runs on
device.

Self-contained: no sibling imports.
"""
import numpy as np


def _build_jax_impl():
    import jax
    import jax.numpy as jnp

    def conv2d(x, w, stride=1):
        kh, kw = w.shape[2], w.shape[3]
        pad = [((kh - 1) // 2,) * 2, ((kw - 1) // 2,) * 2]
        return jax.lax.conv_general_dilated(
            x, w, (stride, stride), pad,
            dimension_numbers=('NCHW', 'OIHW', 'NCHW'))

    def basic_conv(x, w, stride=1):
        return jax.nn.relu(conv2d(x, w, stride))

    def basic_block(x, w1, w2):
        h = jax.nn.relu(conv2d(x, w1))
        h = conv2d(h, w2)
        return jax.nn.relu(x + h)

    def run_layer(x, p, stride):
        x = basic_conv(x, p['down'], stride)
        for (w1, w2) in p['blocks']:
            x = basic_block(x, w1, w2)
        return x

    def bilinear_sample(feat, coords, scale):
        Bb, C, H, W = feat.shape
        xy = coords[..., 0] * scale
        h = jnp.clip(xy[..., 0], 0.0, H - 1.0)
        w = jnp.clip(xy[..., 1], 0.0, W - 1.0)
        h0 = jnp.clip(jnp.floor(h), 0, H - 2).astype(jnp.int32)
        w0 = jnp.clip(jnp.floor(w), 0, W - 2).astype(jnp.int32)
        dh = h - h0
        dw = w - w0

        def per_b(f, h0b, w0b, dhb, dwb):
            f2 = f.reshape(C, H * W)
            i00 = h0b * W + w0b
            v00 = f2[:, i00]; v01 = f2[:, i00 + 1]
            v10 = f2[:, i00 + W]; v11 = f2[:, i00 + W + 1]
            return (v00 * (1 - dhb) * (1 - dwb) + v01 * (1 - dhb) * dwb
                    + v10 * dhb * (1 - dwb) + v11 * dhb * dwb)
        out = jax.vmap(per_b)(feat, h0, w0, dh, dw)
        return out[..., None]

    def voxel_max_pool(pfeat, pind, out_hw, scale):
        Ho, Wo = out_hw
        idx = jnp.floor(pind[..., 0] * scale).astype(jnp.int32)
        hi = jnp.clip(idx[..., 0], 0, Ho - 1)
        wi = jnp.clip(idx[..., 1], 0, Wo - 1)
        flat = hi * Wo + wi

        def per_b(f, s):
            m = jax.ops.segment_max(f.T, s, num_segments=Ho * Wo)
            m = jnp.where(jnp.isfinite(m), m, jnp.zeros_like(m))
            return m.T.reshape(f.shape[0], Ho, Wo)
        return jax.vmap(per_b)(pfeat[..., 0], flat)

    def resize_ac(x, out_hw):
        H, W = x.shape[2], x.shape[3]
        Ho, Wo = out_hw
        if (H, W) == (Ho, Wo):
            return x
        ys = jnp.linspace(0.0, H - 1.0, Ho)
        xs = jnp.linspace(0.0, W - 1.0, Wo)
        y0 = jnp.clip(jnp.floor(ys), 0, H - 2).astype(jnp.int32); wy = ys - y0
        x0 = jnp.clip(jnp.floor(xs), 0, W - 2).astype(jnp.int32); wx = xs - x0
        x_ = (x[:, :, y0, :] * (1 - wy)[None, None, :, None]
              + x[:, :, y0 + 1, :] * wy[None, None, :, None])
        x_ = x_[:, :, :, x0] * (1 - wx) + x_[:, :, :, x0 + 1] * wx
        return x_

    def aux_head(x, w, b):
        return conv2d(x, w) + b[None, :, None, None]

    def forward(c, c_coord_curr, p_coord_curr, deep_64, params):
        c0 = run_layer(c, params['cart_header'], 2)
        pt = bilinear_sample(c0, c_coord_curr, 0.5)
        c0_to_polar = voxel_max_pool(pt, p_coord_curr, (32, 1024), 0.5)
        p0 = run_layer(c0_to_polar, params['polar_header'], 1)
        pt = bilinear_sample(p0, p_coord_curr, 0.5)
        p0_to_cart = voxel_max_pool(pt, c_coord_curr, (256, 256), 0.5)
        c0 = jnp.concatenate([c0, p0_to_cart], axis=1)
        c1 = run_layer(c0, params['cart_res1'], 2)
        pt = bilinear_sample(c1, c_coord_curr, 0.25)
        c1_to_polar = voxel_max_pool(pt, p_coord_curr, (16, 512), 0.25)
        p1 = run_layer(c1_to_polar, params['polar_res1'], 1)
        pt = bilinear_sample(p1, p_coord_curr, 0.25)
        p1_to_cart = voxel_max_pool(pt, c_coord_curr, (128, 128), 0.25)
        c1 = jnp.concatenate([c1, p1_to_cart], axis=1)
        c2 = run_layer(c1, params['cart_res2'], 2)
        c2 = jax.nn.relu(conv2d(c2 + deep_64, params['add_fuse']))
        tgt = (c0.shape[2], c0.shape[3])
        res0 = c0
        res1 = resize_ac(c1, tgt)
        res2 = resize_ac(c2, tgt)
        out = basic_conv(jnp.concatenate([res0, res1, res2], axis=1),
                         params['out_conv1'])
        out = basic_conv(out, params['out_conv2'])
        out_as_point = bilinear_sample(out, c_coord_curr, 0.5)
        polar_res1_as_point = bilinear_sample(p1, p_coord_curr, 0.5)
        res0 = aux_head(res0, params['aux1_w'], params['aux1_b'])
        res1 = aux_head(res1, params['aux2_w'], params['aux2_b'])
        res2 = aux_head(res2, params['aux3_w'], params['aux3_b'])
        return (out_as_point, polar_res1_as_point, res0, res1, res2, c2)

    return forward


_CACHE = {}


def _run_on(dev, args):
    import jax
    import jax.tree_util as jtu
    key = str(dev)
    if key not in _CACHE:
        _CACHE[key] = jax.jit(_build_jax_impl())
    fn = _CACHE[key]
    args = jtu.tree_map(lambda x: jax.device_put(np.asarray(x), dev), args)
    outs = fn(*args)
    return jax.block_until_ready(outs)


def kernel(c, c_coord_curr, p_coord_curr, deep_64, params):
    import jax
    args = (c, c_coord_curr, p_coord_curr, deep_64, params)
    devs = jax.devices()
    try:
        outs = _run_on(devs[0], args)
    except Exception as e:
        # Device compile/run failed; fall back to CPU so the kernel
        # still produces correct results.
        print("kernel: device path failed (%s); falling back to CPU" % type(e).__name__)
        outs = _run_on(jax.devices("cpu")[0], args)
    return tuple(np.asarray(o) for o in outs)
